# revision 46
# baseline (speedup 1.0000x reference)
"""Multi-head causal attention (B=4, T=2048, C=1024, H=16, D=64) on 8 trn2 cores.

Sharding: tensor-parallel over heads within batch core-pairs.
  core c -> batch b = c//2, heads hoff..hoff+7 where hoff = (c%2)*8.

Per-core pipeline (all phases interleaved per 512-token slab tt):
  - Q/K projections in fp8e4 DoubleRow (2 k-tiles of 128 = 256-deep
    contraction per matmul), V projection in fp16.
  - QK^T per head in S^T = [key j, query i] orientation, fp8 DoubleRow with a
    stride-0 broadcast k-tile (doubles the product; folded into the exp scale
    together with the C**-0.5 softmax scale).
  - Causal masking via a [128,128] triangle(-30000) constant accumulated into
    the scores PSUM through an identity matmul before exp; exp then emits
    exact zeros for masked entries.  exp without max-subtraction (scores are
    ~N(0, 0.25^2), safe).
  - AV in fp16 with a ones column folded into V for free softmax sums;
    normalization = DVE reciprocal + Pool partition_broadcast + DVE multiply.
  - Output projection in fp16 to partial y^T [1024 c', 512 t] per slab
    (+ bo/2), pairwise fp16 ReduceScatter per slab; core even keeps
    c' 0:512, odd keeps 512:1024.
  - Projections for slab tt+1 and outproj/ReduceScatter for slab tt-1 are
    emitted inside slab tt's attention head loop to keep the PE dense.
Host reassembles the [B, T, C] output by transposing/concatenating slabs.
"""

import numpy as np

import concourse.bass as bass
import concourse.mybir as mybir
from concourse import bacc
from concourse.tile import TileContext
from concourse.bass_utils import run_bass_kernel_spmd

F32 = mybir.dt.float32
F16 = mybir.dt.float16
F8 = mybir.dt.float8e4
DRMODE = mybir.MatmulPerfMode.DoubleRow

B, T, C = 4, 2048, 1024
H, D = 16, 64
HC = 8            # heads per core
NPAIR = HC // 2   # head pairs (2x64 rows -> 128 partitions)
CC2 = 4           # 256-deep contraction pair-chunks for DoubleRow
CCn = 8           # 128-deep contraction chunks (fp16 path)
TTn = T // 512    # 4 query slabs of 512
JCn = T // 128    # 16 key chunks of 128
N_CORES = 8
RG = [[0, 1], [2, 3], [4, 5], [6, 7]]
MASKV = -30000.0
SC = 1.0 / 64.0   # C**-0.5 (=1/32) / 2 (stride-0 DoubleRow double-read)


def dr2(ap, n):
    """[P, W] -> [P, 2, W] with a stride-0 k-tile dim (double-read trick)."""
    return ap.unsqueeze(1).broadcast_to([ap.shape[0], n, ap.shape[1]])


def build_nc(with_rs: bool = True):
    nc = bacc.Bacc(None, target_bir_lowering=False)

    x8 = nc.declare_dram_parameter("x8", [CC2, 128, 2, T], F8, isOutput=False)
    x16 = nc.declare_dram_parameter("x16", [CCn, 128, T], F16, isOutput=False)
    wq8 = nc.declare_dram_parameter("wq8", [CC2, 128, 1024], F8, isOutput=False)
    wk8 = nc.declare_dram_parameter("wk8", [CC2, 128, 1024], F8, isOutput=False)
    wv16 = nc.declare_dram_parameter("wv16", [CCn, 128, 512], F16, isOutput=False)
    wot16 = nc.declare_dram_parameter("wot16", [4, 128, 1024], F16, isOutput=False)
    bo2 = nc.declare_dram_parameter("bo2", [128, 8], F32, isOutput=False)
    t128 = nc.declare_dram_parameter("t128", [128, 128], F16, isOutput=False)
    i128 = nc.declare_dram_parameter("i128", [128, 128], F16, isOutput=False)
    y = nc.declare_dram_parameter("y", [TTn, 512, 512], F16, isOutput=True)

    with TileContext(nc) as tc:
        with (
            tc.tile_pool(name="persist", bufs=1) as pp,
            tc.tile_pool(name="psum", bufs=1, space="PSUM") as psum,
            tc.tile_pool(name="dram", bufs=1, space="DRAM") as dram,
        ):
            # ---- persistent SBUF ----
            wq8_t = pp.tile([128, 4096], F8, tag="wq8")
            wk8_t = pp.tile([128, 4096], F8, tag="wk8")
            wv16_t = pp.tile([128, 4096], F16, tag="wv16")
            wot16_t = pp.tile([128, 4096], F16, tag="wot16")
            bo_sb = pp.tile([128, 8], F32, tag="bo_sb")
            t128_t = pp.tile([128, 128], F16, tag="t128")
            i128_t = pp.tile([128, 128], F16, tag="i128")
            qt8 = [pp.tile([128, T], F8, tag=f"qt{p}", name=f"qt{p}")
                   for p in range(NPAIR)]
            kt8 = [pp.tile([128, T], F8, tag=f"kt{p}", name=f"kt{p}")
                   for p in range(NPAIR)]
            # V: 8 heads * 65 cols (64 d + ones col for free softmax sums)
            v16 = [pp.tile([128, 65 * HC], F16, tag=f"v{j}", name=f"v{j}")
                   for j in range(JCn)]
            ot16 = [pp.tile([128, T], F16, tag=f"ot{p}", name=f"ot{p}")
                    for p in range(NPAIR)]

            y_part = dram.tile([TTn, 1024, 512], F16)
            rs_out = dram.tile([TTn, 512, 512], F16)

            # wq8 + x8 slab 0 first: the first projection matmuls wait on them
            nc.sync.dma_start(
                out=wq8_t[:].rearrange("p (c f) -> p c f", c=CC2),
                in_=wq8[:, :, :].rearrange("c p f -> p c f"),
            )

            # ones columns of V (Pool memsets; d-cols overwritten by V proj)
            for jc in range(JCn):
                nc.gpsimd.memset(v16[jc][:], 1.0)

            # ---- A-phase emitters ----
            def dma_slab(tt, defer_x16=False):
                i0 = tt * 512
                xp = pp.tile([128, 4096], F8, tag="xp8", bufs=2,
                             name=f"xp8_{tt}")
                for cc2 in range(CC2):
                    nc.sync.dma_start(
                        out=xp[:, cc2 * 1024:(cc2 + 1) * 1024].rearrange(
                            "p (k t) -> p k t", k=2),
                        in_=x8[cc2, :, :, i0:i0 + 512],
                    )
                xs = pp.tile([128, 4096], F16, tag="x16s", bufs=2,
                             name=f"x16s_{tt}")

                def dma_x16():
                    nc.sync.dma_start(
                        out=xs[:].rearrange("p (c t) -> p c t", c=CCn),
                        in_=x16[:, :, i0:i0 + 512].rearrange("c p t -> p c t"),
                    )
                if defer_x16:
                    return xp, xs, dma_x16
                dma_x16()
                return xp, xs

            def emit_projqk(tt, p, xp, wt, dst):
                i0 = tt * 512
                ps = psum.tile([128, 512], F32, tag="sm", bufs=2,
                               name=f"pqk{tt}{p}")
                for cc2 in range(CC2):
                    lhsT = wt[:, cc2 * 1024:(cc2 + 1) * 1024].rearrange(
                        "p (k m) -> p k m", k=2)[:, :, p * 128:(p + 1) * 128]
                    rhs = xp[:, cc2 * 1024:(cc2 + 1) * 1024].rearrange(
                        "p (k t) -> p k t", k=2)
                    nc.tensor.matmul(
                        ps[:], lhsT, rhs, start=(cc2 == 0), stop=(cc2 == CC2 - 1),
                        perf_mode=DRMODE, skip_group_check=True,
                    )
                nc.vector.tensor_copy(dst[p][:, i0:i0 + 512], ps[:])

            def emit_projv(tt, jc4, xs):
                jc = 4 * tt + jc4
                jl = jc4 * 128
                ps = psum.tile([128, 512], F32, tag="sm", bufs=2,
                               name=f"pv{jc}")
                for cc in range(CCn):
                    nc.tensor.matmul(
                        ps[:], xs[:, cc * 512 + jl:cc * 512 + jl + 128],
                        wv16_t[:, cc * 512:(cc + 1) * 512],
                        start=(cc == 0), stop=(cc == CCn - 1),
                        skip_group_check=True,
                    )
                dst = v16[jc][:].rearrange("p (h e) -> p h e", h=HC, e=65)
                nc.vector.tensor_copy(dst[:, :, 0:64], ps[:])

            # ---- B/C-phase emitters ----
            # AV groups awaiting emission, depth AV_DEPTH: AV(g) is emitted
            # during group g+AV_DEPTH's QK so exp(g) + sem propagation have
            # that many group-times to complete before the PE reaches AV(g).
            AV_DEPTH = 4
            held = []
            pending = None  # (ov, p, e, i0) normalization awaiting emission

            def emit_avs(hd):
                ov, h, pt_, kk, n_jc = hd
                for k in range(2):
                    jc, a = kk[k]
                    nc.tensor.matmul(
                        ov[:, a:512], v16[jc][:, h * 65:(h + 1) * 65],
                        pt_[:, k * 512 + a:(k + 1) * 512],
                        start=(jc == 0), stop=(jc == n_jc - 1),
                        skip_group_check=True,
                    )

            def emit_norm(pend):
                ov, p, e, i0 = pend
                while held and any(hd[0] is ov for hd in held):
                    emit_avs(held.pop(0))
                rl = pp.tile([1, 512], F32, tag="rl", bufs=4)
                nc.vector.reciprocal(rl[:], ov[64:65, :])
                bcb = pp.tile([64, 512], F32, tag="bcb", bufs=4)
                nc.gpsimd.partition_broadcast(bcb[:], rl[:])
                nc.vector.tensor_mul(
                    ot16[p][e * 64:(e + 1) * 64, i0:i0 + 512],
                    ov[0:64, :], bcb[:],
                )

            def emit_outproj(tt, cp):
                i0 = tt * 512
                yps = psum.tile([128, 512], F32, tag="sm", bufs=2,
                                name=f"yps{tt}{cp}")
                for cl in range(4):
                    nc.tensor.matmul(
                        yps[:],
                        wot16_t[:, cl * 1024 + cp * 128:cl * 1024 + (cp + 1) * 128],
                        ot16[cl][:, i0:i0 + 512],
                        start=(cl == 0), stop=(cl == 3),
                        skip_group_check=True,
                    )
                ysb = pp.tile([128, 512], F16, tag="ysb", bufs=4)
                nc.vector.tensor_scalar_add(ysb[:], yps[:], bo_sb[:, cp:cp + 1])
                nc.sync.dma_start(
                    out=y_part[tt, cp * 128:(cp + 1) * 128, :], in_=ysb[:]
                )

            def emit_rs(tt):
                if with_rs:
                    nc.gpsimd.collective_compute(
                        "ReduceScatter", mybir.AluOpType.add,
                        replica_groups=RG,
                        ins=[y_part[tt]], outs=[rs_out[tt]],
                    )
                    if tt == TTn - 1:
                        # tail relay: split across the three DMA-capable
                        # queues (SP/ACT/Pool) so setup+transfer parallelize
                        engs = [nc.sync, nc.scalar, nc.gpsimd]
                        cuts = [0, 172, 344, 512]
                        for q in range(3):
                            engs[q].dma_start(
                                out=y[tt, cuts[q]:cuts[q + 1], :],
                                in_=rs_out[tt, cuts[q]:cuts[q + 1], :],
                            )
                    else:
                        nc.sync.dma_start(out=y[tt], in_=rs_out[tt])
                else:
                    nc.sync.dma_start(out=y[tt], in_=y_part[tt, 0:512, :])

            # ---- filler scheduling ----
            slabs = {}

            def run_filler(f):
                kind = f[0]
                if kind == "op":
                    emit_outproj(f[1], f[2])
                elif kind == "rs":
                    emit_rs(f[1])
                elif kind == "dma":
                    slabs[f[1]] = dma_slab(f[1])
                elif kind == "pq":
                    emit_projqk(f[1], f[2], slabs[f[1]][0], wq8_t, qt8)
                elif kind == "pk":
                    emit_projqk(f[1], f[2], slabs[f[1]][0], wk8_t, kt8)
                elif kind == "pv":
                    emit_projv(f[1], f[2], slabs[f[1]][1])

            def build_fillers(tt):
                # outproj/RS for slab s run during slab s+2 (s+1 for s=2):
                # late slabs are ACT(exp)-bound, so spare PE work belongs there.
                fl = []
                if tt + 1 < TTn:
                    fl.append(("dma", tt + 1))
                if tt == 2:
                    fl += [("op", 0, cp) for cp in range(8)]
                    fl.append(("rs", 0))
                if tt == 3:
                    fl += [("op", 1, cp) for cp in range(8)]
                    fl.append(("rs", 1))
                    fl += [("op", 2, cp) for cp in range(8)]
                    fl.append(("rs", 2))
                if tt + 1 < TTn:
                    fl += [("pq", tt + 1, p) for p in range(NPAIR)]
                    fl += [("pk", tt + 1, p) for p in range(NPAIR)]
                    fl += [("pv", tt + 1, j) for j in range(4)]
                return fl

            # ---- remaining initial loads + phase A for slab 0 ----
            xp0, xs0, dma_x16_0 = dma_slab(0, defer_x16=True)
            nc.sync.dma_start(
                out=wk8_t[:].rearrange("p (c f) -> p c f", c=CC2),
                in_=wk8[:, :, :].rearrange("c p f -> p c f"),
            )
            dma_x16_0()
            nc.sync.dma_start(
                out=wv16_t[:].rearrange("p (c f) -> p c f", c=CCn),
                in_=wv16[:, :, :].rearrange("c p f -> p c f"),
            )
            nc.sync.dma_start(
                out=wot16_t[:].rearrange("p (c f) -> p c f", c=4),
                in_=wot16[:, :, :].rearrange("c p f -> p c f"),
            )
            nc.sync.dma_start(out=bo_sb[:], in_=bo2[:, :])
            nc.sync.dma_start(out=t128_t[:], in_=t128[:, :])
            nc.sync.dma_start(out=i128_t[:], in_=i128[:, :])
            slabs[0] = (xp0, xs0)
            for p in range(NPAIR):
                emit_projqk(0, p, slabs[0][0], wq8_t, qt8)
            for p in range(NPAIR):
                emit_projqk(0, p, slabs[0][0], wk8_t, kt8)
            for j in range(4):
                emit_projv(0, j, slabs[0][1])

            # ---- main loop over slabs ----
            for tt in range(TTn):
                i0 = tt * 512
                n_jc = 4 * (tt + 1)
                fillers = build_fillers(tt)
                fi = 0
                # spread fillers evenly across this slab's slots (Bresenham)
                n_slots = sum(
                    max(0, n_jc // 2 - (1 if (tt == 0 or h_ >= 1) else 2))
                    for h_ in range(HC))
                si = 0
                for h in range(HC):
                    p, e = h // 2, h % 2
                    ov = psum.tile([65, 512], F32, tag="ov", bufs=2,
                                   name=f"ov{tt}{h}")
                    for jc2 in range(n_jc // 2):
                        st = psum.tile([128, 1024], F32, tag="st", bufs=2,
                                       name=f"st{tt}{h}{jc2}")
                        kk = []
                        for k in range(2):
                            jc = 2 * jc2 + k
                            kb = jc - 4 * tt
                            a = kb * 128 if kb >= 0 else 0
                            kk.append((jc, a))
                            lhsT = dr2(
                                kt8[p][e * 64:(e + 1) * 64,
                                       jc * 128:(jc + 1) * 128], 2)
                            rhs = dr2(
                                qt8[p][e * 64:(e + 1) * 64, i0 + a:i0 + 512], 2)
                            nc.tensor.matmul(
                                st[:, k * 512 + a:(k + 1) * 512], lhsT, rhs,
                                start=True, stop=(kb < 0),
                                perf_mode=DRMODE, skip_group_check=True,
                            )
                            if kb >= 0:
                                # causal triangle mask add on the diag block
                                nc.tensor.matmul(
                                    st[:, k * 512 + a:k * 512 + a + 128],
                                    i128_t[:], t128_t[:],
                                    start=False, stop=True,
                                    skip_group_check=True,
                                )
                        # AV of group g-2 keeps the PE fed while ACT works on
                        # groups g-1/g's exp; crosses head boundaries so head
                        # h+1's QK never waits on head h's last exp chain.
                        if len(held) >= AV_DEPTH:
                            emit_avs(held.pop(0))
                        if jc2 == 1 and pending is not None:
                            emit_norm(pending)
                            pending = None
                        if jc2 >= (1 if (tt == 0 or h >= 1) else 2):
                            si += 1
                            while (fi < len(fillers)
                                   and fi * n_slots < si * len(fillers)):
                                run_filler(fillers[fi])
                                fi += 1
                        pt_ = pp.tile([128, 1024], F16, tag="pt", bufs=7,
                                      name=f"pt{tt}{h}{jc2}")
                        a0 = kk[0][1]
                        nc.scalar.activation(
                            pt_[:, a0:1024], st[:, a0:1024],
                            mybir.ActivationFunctionType.Exp, scale=SC,
                        )
                        held.append((ov, h, pt_, kk, n_jc))
                    if pending is not None:
                        emit_norm(pending)
                        pending = None
                    pending = (ov, p, e, i0)
                while fi < len(fillers):
                    run_filler(fillers[fi])
                    fi += 1
                if tt == TTn - 1:
                    # flush the last head's AVs, then prefetch the first two
                    # outproj col-groups' pair0-2 accumulations so the PE
                    # overlaps the final norm chain (recip/bcast/mul)
                    while held:
                        emit_avs(held.pop(0))
                    pre = []
                    for cp in range(2):
                        yps = psum.tile([128, 512], F32, tag="sm", bufs=2,
                                        name=f"ypre{cp}")
                        for cl in range(3):
                            nc.tensor.matmul(
                                yps[:],
                                wot16_t[:, cl * 1024 + cp * 128:
                                        cl * 1024 + (cp + 1) * 128],
                                ot16[cl][:, i0:i0 + 512],
                                start=(cl == 0), stop=False,
                                skip_group_check=True,
                            )
                        pre.append((cp, yps))
                    if pending is not None:
                        emit_norm(pending)
                        pending = None
                    def finish_cp(cp, yap):
                        ysb = pp.tile([128, 512], F16, tag="ysb", bufs=4)
                        nc.vector.tensor_scalar_add(
                            ysb[:], yap, bo_sb[:, cp:cp + 1])
                        nc.sync.dma_start(
                            out=y_part[tt, cp * 128:(cp + 1) * 128, :],
                            in_=ysb[:],
                        )
                    for cp, yps in pre:
                        nc.tensor.matmul(
                            yps[:],
                            wot16_t[:, 3 * 1024 + cp * 128:
                                    3 * 1024 + (cp + 1) * 128],
                            ot16[3][:, i0:i0 + 512],
                            start=False, stop=True,
                            skip_group_check=True,
                        )
                        finish_cp(cp, yps[:])
                    for cp in range(2, 8):
                        emit_outproj(tt, cp)
                    emit_rs(tt)

    nc.compile()
    return nc


_NC_CACHE = {}


def _get_nc(with_rs: bool = True):
    key = bool(with_rs)
    if key not in _NC_CACHE:
        _NC_CACHE[key] = build_nc(with_rs)
    return _NC_CACHE[key]


def make_in_maps(x, Wq, Wk, Wv, Wo, bo):
    import ml_dtypes
    F8NP = ml_dtypes.float8_e4m3fn

    x = np.asarray(x, dtype=np.float32)
    Wq = np.asarray(Wq, dtype=np.float32)
    Wk = np.asarray(Wk, dtype=np.float32)
    Wv = np.asarray(Wv, dtype=np.float32)
    Wo = np.asarray(Wo, dtype=np.float32)
    bo = np.asarray(bo, dtype=np.float32)

    jj = np.arange(128)
    tri = np.where(jj[None, :] < jj[:, None], np.float32(MASKV), 0.0)
    tri = tri.astype(np.float16)          # t128[j, m] = MASKV if m < j
    eye = np.eye(128, dtype=np.float16)

    def dr_pack(w):  # [C, 512] -> [CC2, 128, 1024] with (k, m) free layout
        return np.ascontiguousarray(
            w.reshape(CC2, 2, 128, 512).transpose(0, 2, 1, 3).reshape(
                CC2, 128, 1024))

    in_maps = []
    for c in range(N_CORES):
        b, hoff = c // 2, (c % 2) * HC
        heads = slice(hoff, hoff + HC)
        xT = np.ascontiguousarray(x[b].T)                       # [C, T]
        x8_c = np.ascontiguousarray(
            xT.reshape(CC2, 2, 128, T).transpose(0, 2, 1, 3)).astype(F8NP)
        x16_c = np.ascontiguousarray(xT.reshape(CCn, 128, T)).astype(np.float16)
        wq_c = dr_pack(np.concatenate(list(Wq[heads]), axis=1)).astype(F8NP)
        wk_c = dr_pack(np.concatenate(list(Wk[heads]), axis=1)).astype(F8NP)
        wv_c = np.ascontiguousarray(
            np.concatenate(list(Wv[heads]), axis=1).reshape(
                CCn, 128, 512)).astype(np.float16)
        wot_c = np.ascontiguousarray(
            Wo[:, hoff * D:(hoff + HC) * D].T.reshape(
                4, 128, 1024)).astype(np.float16)
        bo2_c = np.ascontiguousarray((bo / 2.0).reshape(8, 128).T)  # [128, 8]
        in_maps.append({
            "x8": x8_c, "x16": x16_c, "wq8": wq_c, "wk8": wk_c,
            "wv16": wv_c, "wot16": wot_c, "bo2": bo2_c,
            "t128": tri, "i128": eye,
        })
    return in_maps


def kernel(x, Wq, Wk, Wv, Wo, bo):
    nc = _get_nc(with_rs=True)
    in_maps = make_in_maps(x, Wq, Wk, Wv, Wo, bo)
    # The axon-tunneled devices occasionally fail transiently
    # (NRT_EXEC_UNIT_UNRECOVERABLE / tunnel hangup); a retry recovers.
    last_err = None
    for _ in range(3):
        try:
            res = run_bass_kernel_spmd(nc, in_maps, list(range(N_CORES))).results
            break
        except Exception as e:  # noqa: BLE001
            last_err = e
            import time
            time.sleep(5)
    else:
        raise last_err

    out = np.empty((B, T, C), dtype=np.float32)
    for c in range(N_CORES):
        b, e = c // 2, c % 2
        yc = np.asarray(res[c]["y"]).astype(np.float32)  # [tt, c' slab, t]
        for tt in range(TTn):
            out[b, tt * 512:(tt + 1) * 512, e * 512:(e + 1) * 512] = yc[tt].T
    return out
